# revision 18
# baseline (speedup 1.0000x reference)
"""Trainium2 Bass kernel for nn_DAC_558345749225 (dynamic rotated grouped conv).

Reference (per sample b):
  pooled = mean_{H,W} x[b]                                  [C]
  angles = tanh(relu(pooled@W1^T+b1)@W2^T+b2) * pi/4        [G]
  rot[g] = bilinear-rotate(base_kernel[g], angles[g])       [Cg,Cg,3,3]
  feat   = grouped_conv3x3(x[b], rot, groups=G, pad=1)
  mod    = sigmoid(relu(pooled@M1^T+bm1)@M2^T+bm2)          [C]
  out    = feat * mod[:,None,None]

Sharding: data-parallel over batch — 2 samples per core on 8 cores.

Per-core pipeline (samples b=0,1; packs p=0,1 of 4 groups each):
  - x is zero-padded to 66x66 on the host and DMAd contiguously into SBUF
    tiles [128ch, 4356] declared float32r (raw fp32 bits; the PE rounds
    internally — verified bitwise-identical to pre-rounded inputs).
  - pooling: free-dim reduce over the padded row (border zeros don't change
    the sum); the 1/4096 scale is folded into the MLP weights host-side.
    Sample 0 reduces on DVE, sample 1 on GpSimd so they run concurrently.
  - per-sample: tiny MLPs on PE + ACT (Relu/Tanh/Sigmoid, cos via
    Sin(x+pi/2)); the 9x9 rotation map R[ij,mn](theta) built with ~22
    elementwise DVE ops on an [8=(g), 81=(mn,ij)] layout.
  - per (b,pack): R scattered into block-diag A [36,36]; one fp32 matmul
      out1[(g,ij),(ci,co)] = A^T @ Bmat   (Bmat = host-rearranged base_kernel)
    out1 rounded to float32r by the ACT copy out of PSUM, then 36 small sync
    DMAs scatter it into block-diag conv weights lhsT [128=(g,ci), 9*128].
  - conv: per (b,pack), 8 chunks of 8 output rows; 9 shift matmuls (float32r,
    1 PE cycle/row, 512 moving rows) accumulate into one PSUM bank; epilogue
    on ACT multiplies by the sigmoid gate (per-partition scale) and the
    result is DMAd to DRAM.
"""
import math
import numpy as np

N_CORES = 8
B, C, H, W = 16, 256, 64, 64
G, Cg = 8, 32
HID = 64
Bc = B // N_CORES          # samples per core = 2
NU = Bc * 2                # (b, pack) units per core = 4
HP, WP = H + 2, W + 2      # padded 66 x 66
NPIX = HP * WP             # 4356

_CACHE = {}


def _host_consts(base_kernel, ap_w1, ap_b1, ap_w2, ap_b2,
                 meta_w1, meta_b1, meta_w2, meta_b2):
    f32 = np.float32
    # Bmat [2, 36, 1024]: [p][g4*9+mn][ci*32+co] = base[4p+g4, co, ci, m, n]
    bk = np.asarray(base_kernel, f32)                      # [G, co, ci, 3, 3]
    bm = np.transpose(bk, (0, 3, 4, 2, 1))                 # [G, m, n, ci, co]
    bmat = np.ascontiguousarray(bm.reshape(2, 36, Cg * Cg))

    # R-build constants over free layout f = mn*9 + ij (mn-major)
    f = np.arange(81)
    mn, ij = f // 9, f % 9
    i, j = ij // 3, ij % 3
    m, n = mn // 3, mn % 3
    blocks = [
        (j - 1).astype(f32),                               # xx
        (i - 1).astype(f32),                               # yy
        n.astype(f32),                                     # nv
        m.astype(f32),                                     # mv
    ]
    consts = np.tile(np.concatenate(blocks)[None, :], (16, 1)).astype(f32)

    scale = f32(1.0 / (H * W))
    w1T = np.asarray(ap_w1, f32).T * scale                 # [256, 64]
    m1T = np.asarray(meta_w1, f32).T * scale
    mlp1 = np.ascontiguousarray(np.concatenate(
        [w1T[:128], w1T[128:], m1T[:128], m1T[128:]], axis=1), f32)  # [128,256]
    w2T = np.asarray(ap_w2, f32).T                         # [64, 8]
    m2T = np.asarray(meta_w2, f32).T                       # [64, 256]
    mlp2 = np.ascontiguousarray(np.concatenate([w2T, m2T], axis=1), f32)
    bias64 = np.ascontiguousarray(
        np.stack([np.asarray(ap_b1, f32), np.asarray(meta_b1, f32)], axis=1))
    b2v = np.asarray(ap_b2, f32).reshape(8, 1).copy()
    mb2v = np.ascontiguousarray(np.asarray(meta_b2, f32).reshape(2, 128).T)
    return dict(bmat=bmat, consts=consts, mlp1=mlp1, mlp2=mlp2,
                bias64=bias64, b2v=b2v, mb2v=mb2v)


def _build_nc():
    import concourse.bass as bass
    import concourse.tile as tile
    from concourse import bacc, mybir
    dt = mybir.dt
    AF = mybir.ActivationFunctionType
    OP = mybir.AluOpType

    nc = bacc.Bacc("TRN2", target_bir_lowering=False, debug=False,
                   enable_asserts=False, num_devices=N_CORES)

    xs = nc.dram_tensor("xs", [Bc, C, HP, WP], dt.float32r, kind="ExternalInput").ap()
    bmat_d = nc.dram_tensor("bmat", [2, 36, 1024], dt.float32, kind="ExternalInput").ap()
    consts_d = nc.dram_tensor("consts", [16, 324], dt.float32, kind="ExternalInput").ap()
    mlp1_d = nc.dram_tensor("mlp1", [128, 256], dt.float32, kind="ExternalInput").ap()
    mlp2_d = nc.dram_tensor("mlp2", [64, 264], dt.float32, kind="ExternalInput").ap()
    bias64_d = nc.dram_tensor("bias64", [64, 2], dt.float32, kind="ExternalInput").ap()
    b2v_d = nc.dram_tensor("b2v", [8, 1], dt.float32, kind="ExternalInput").ap()
    mb2v_d = nc.dram_tensor("mb2v", [128, 2], dt.float32, kind="ExternalInput").ap()
    y = nc.dram_tensor("y", [Bc, C, H, W], dt.float32, kind="ExternalOutput").ap()

    xs_flat = xs.rearrange("b c h w -> (b c) (h w)")
    y_flat = y.rearrange("b c h w -> (b c) (h w)")

    with tile.TileContext(nc) as tc:
        from contextlib import ExitStack
        ctx = ExitStack()
        cpool = ctx.enter_context(tc.tile_pool(name="cpool", bufs=1))
        xpool = ctx.enter_context(tc.tile_pool(name="xpool", bufs=NU))
        wpool = ctx.enter_context(tc.tile_pool(name="wpool", bufs=NU))
        o1pool = ctx.enter_context(tc.tile_pool(name="o1pool", bufs=2))
        apool = ctx.enter_context(tc.tile_pool(name="apool", bufs=2))
        outpool = ctx.enter_context(tc.tile_pool(name="outpool", bufs=5))
        pconv = ctx.enter_context(tc.tile_pool(name="pconv", bufs=6, space="PSUM"))
        psmall = ctx.enter_context(tc.tile_pool(name="psmall", bufs=2, space="PSUM"))

        # ---------- constants ----------
        consts_t = cpool.tile([16, 324], dt.float32)
        nc.sync.dma_start(consts_t[:], consts_d[:])
        XX, YY, NV, MV = (consts_t[:, 81 * k:81 * (k + 1)] for k in range(4))
        mlp1_t = cpool.tile([128, 256], dt.float32)
        nc.sync.dma_start(mlp1_t[:], mlp1_d[:])
        mlp2_t = cpool.tile([64, 264], dt.float32)
        nc.sync.dma_start(mlp2_t[:], mlp2_d[:])
        bias64_t = cpool.tile([64, 2], dt.float32)
        nc.sync.dma_start(bias64_t[:], bias64_d[:])
        b2v_t = cpool.tile([8, 1], dt.float32)
        nc.sync.dma_start(b2v_t[:], b2v_d[:])
        mb2v_t = cpool.tile([128, 2], dt.float32)
        nc.sync.dma_start(mb2v_t[:], mb2v_d[:])
        bsb = cpool.tile([36, 2048], dt.float32)
        nc.sync.dma_start(bsb[:, 0:1024], bmat_d[0])
        nc.sync.dma_start(bsb[:, 1024:2048], bmat_d[1])
        halfpi = cpool.tile([8, 1], dt.float32)
        nc.gpsimd.memset(halfpi[:], math.pi / 2)

        # ---------- x loads (contiguous, two halves per tile) ----------
        HH = NPIX // 2                 # 2178
        x_tiles = []
        x_dma_insts = []
        for u in range(NU):
            b, p = divmod(u, 2)
            xt = xpool.tile([128, NPIX], dt.float32r)
            src = xs_flat[b * C + 128 * p:b * C + 128 * (p + 1), :]
            i0 = nc.sync.dma_start(xt[:, 0:HH], src[:, 0:HH])
            i1 = nc.sync.dma_start(xt[:, HH:NPIX], src[:, HH:NPIX])
            x_tiles.append(xt)
            x_dma_insts.append((i0, i1))
        # serialize sample-1 loads behind sample-0 so sample 0 gets full BW
        if False:
            for u in (2, 3):
                for i_late in x_dma_insts[u]:
                    for u0 in (0, 1):
                        for i_early in x_dma_insts[u0]:
                            tile.add_dep_helper(i_late.ins, i_early.ins, sync=True,
                                                reason="sample0-dma-first")

        # ---------- pooling (emitted per-sample inside the loop below) ----------
        # pooled col layout: col = 2*pack + b
        pooled = cpool.tile([128, NU], dt.float32)
        pp = cpool.tile([128, 2 * NU], dt.float32)

        def emit_pooling(b):
            for p in range(2):
                u = 2 * b + p
                xf = x_tiles[u][:].bitcast(dt.float32)
                nc.vector.reduce_sum(pp[:, 2 * u:2 * u + 1], xf[:, 0:HH],
                                     axis=mybir.AxisListType.X)
                nc.vector.reduce_sum(pp[:, 2 * u + 1:2 * u + 2], xf[:, HH:NPIX],
                                     axis=mybir.AxisListType.X)
                nc.vector.tensor_tensor(pooled[:, 2 * p + b:2 * p + b + 1],
                                        pp[:, 2 * u:2 * u + 1],
                                        pp[:, 2 * u + 1:2 * u + 2], op=OP.add)

        # ---------- per-sample MLPs + R build ----------
        mod_sb = cpool.tile([128, NU], dt.float32)   # col = 2*pack + b

        def vt(nm):
            return cpool.tile([8, 81], dt.float32, name=nm)

        TT = nc.vector.tensor_tensor
        TS = nc.vector.tensor_scalar
        STT = nc.vector.scalar_tensor_tensor

        lts = {}
        for b in range(Bc):
            emit_pooling(b)
            # --- angle MLP (this sample only) ---
            h_ps = psmall.tile([64, 1], dt.float32, tag="mlp", name=f"hps{b}")
            nc.tensor.matmul(h_ps[:], mlp1_t[:, 0:64], pooled[:, b:b + 1],
                             start=True, stop=False)
            nc.tensor.matmul(h_ps[:], mlp1_t[:, 64:128], pooled[:, 2 + b:3 + b],
                             start=False, stop=True)
            h_sb = cpool.tile([64, 1], dt.float32, name=f"hsb{b}")
            nc.scalar.activation(h_sb[:], h_ps[:], AF.Relu, bias=bias64_t[:, 0:1])
            ang_ps = psmall.tile([8, 1], dt.float32, tag="mlp", name=f"aps{b}")
            nc.tensor.matmul(ang_ps[:], mlp2_t[:, 0:8], h_sb[:], start=True, stop=True)
            ang_t = cpool.tile([8, 1], dt.float32, name=f"angt{b}")
            nc.scalar.activation(ang_t[:], ang_ps[:], AF.Tanh, bias=b2v_t[:])
            ang_sb = cpool.tile([8, 1], dt.float32, name=f"angs{b}")
            nc.vector.tensor_scalar_mul(ang_sb[:], ang_t[:], math.pi / 4)
            c_sb = cpool.tile([8, 1], dt.float32, name=f"csb{b}")
            nc.scalar.activation(c_sb[:], ang_sb[:], AF.Sin, bias=halfpi[:])
            s_sb = cpool.tile([8, 1], dt.float32, name=f"ssb{b}")
            nc.scalar.activation(s_sb[:], ang_sb[:], AF.Sin)

            # --- R build on an [8=(g), 81=(mn,ij)] layout ---
            # scratch tiles shared across samples (Tile serializes the WAR)
            txc, tys, xr, av, fx, avp = (vt(f"rx{k}") for k in range(6))
            tyc, yr, bv, fy, bvp = (vt(f"ry{k}") for k in range(5))
            u0t, u1t, uu, v0t, v1t, vv = (vt(f"ru{k}") for k in range(6))
            r_b = vt(f"rall_{b}")
            X8, Y8, N8, M8 = XX[0:8], YY[0:8], NV[0:8], MV[0:8]
            cS, sS = c_sb[:], s_sb[:]
            nc.vector.tensor_scalar_mul(txc[:], X8, cS)
            nc.vector.tensor_scalar_mul(tys[:], Y8, sS)
            TT(xr[:], txc[:], tys[:], op=OP.add)
            TS(av[:], xr[:], 0.0, None, op0=OP.is_ge)
            STT(fx[:], xr[:], 1.0, av[:], op0=OP.add, op1=OP.subtract)
            nc.vector.tensor_scalar_add(avp[:], av[:], 1.0)
            nc.vector.tensor_scalar_mul(txc[:], X8, sS)
            nc.vector.tensor_scalar_mul(tyc[:], Y8, cS)
            TT(yr[:], tyc[:], txc[:], op=OP.subtract)
            TS(bv[:], yr[:], 0.0, None, op0=OP.is_ge)
            STT(fy[:], yr[:], 1.0, bv[:], op0=OP.add, op1=OP.subtract)
            nc.vector.tensor_scalar_add(bvp[:], bv[:], 1.0)
            TT(u0t[:], N8, av[:], op=OP.is_equal)
            TT(u1t[:], N8, avp[:], op=OP.is_equal)
            TT(uu[:], u1t[:], u0t[:], op=OP.subtract)
            TT(uu[:], uu[:], fx[:], op=OP.mult)
            TT(uu[:], uu[:], u0t[:], op=OP.add)
            TT(v0t[:], M8, bv[:], op=OP.is_equal)
            TT(v1t[:], M8, bvp[:], op=OP.is_equal)
            TT(vv[:], v1t[:], v0t[:], op=OP.subtract)
            TT(vv[:], vv[:], fy[:], op=OP.mult)
            TT(vv[:], vv[:], v0t[:], op=OP.add)
            TT(r_b[:], uu[:], vv[:], op=OP.mult)

            # --- rotation matmul + weight scatter for this sample's 2 packs ---
            for p in range(2):
                u = 2 * b + p
                a_t = apool.tile([36, 36], dt.float32, name=f"a{u}")
                nc.gpsimd.memset(a_t[:], 0.0)
                for g4 in range(4):
                    r = 4 * p + g4
                    nc.sync.dma_start(
                        a_t[:][9 * g4:9 * (g4 + 1), 9 * g4:9 * (g4 + 1)],
                        r_b[:][r:r + 1].rearrange("q (mn ij) -> q mn ij", ij=9))
                o1_t = o1pool.tile([36, 1024], dt.float32r, name=f"o1{u}")
                for hh in range(2):
                    rot_ps = psmall.tile([36, 512], dt.float32, tag="mlp",
                                         name=f"rps{u}{hh}")
                    nc.tensor.matmul(
                        rot_ps[:], a_t[:],
                        bsb[:, 1024 * p + 512 * hh:1024 * p + 512 * (hh + 1)],
                        start=True, stop=True)
                    nc.scalar.copy(o1_t[:, 512 * hh:512 * (hh + 1)], rot_ps[:])
                lt = wpool.tile([128, 9 * 128], dt.float32r, name=f"lt{u}")
                lts[u] = lt
                nc.gpsimd.memset(lt[:].bitcast(dt.float32), 0.0)
                for g4 in range(4):
                    srcv = o1_t[:][9 * g4:9 * (g4 + 1)].rearrange(
                        "q (ci co) -> q ci co", co=32)
                    for ij in range(9):
                        nc.sync.dma_start(
                            lt[:][32 * g4:32 * (g4 + 1),
                                  128 * ij + 32 * g4:128 * ij + 32 * (g4 + 1)],
                            srcv[ij:ij + 1])

            # --- gate MLP (emitted after rotation; PE order still fine) ---
            m_ps = psmall.tile([64, 1], dt.float32, tag="mlp", name=f"mps{b}")
            nc.tensor.matmul(m_ps[:], mlp1_t[:, 128:192], pooled[:, b:b + 1],
                             start=True, stop=False)
            nc.tensor.matmul(m_ps[:], mlp1_t[:, 192:256], pooled[:, 2 + b:3 + b],
                             start=False, stop=True)
            m_sb = cpool.tile([64, 1], dt.float32, name=f"msb{b}")
            nc.scalar.activation(m_sb[:], m_ps[:], AF.Relu, bias=bias64_t[:, 1:2])
            for p in range(2):
                mod_ps = psmall.tile([128, 1], dt.float32, tag="mlp",
                                     name=f"modps{b}{p}")
                nc.tensor.matmul(mod_ps[:],
                                 mlp2_t[:, 8 + 128 * p:8 + 128 * (p + 1)],
                                 m_sb[:], start=True, stop=True)
                nc.scalar.activation(mod_sb[:, 2 * p + b:2 * p + b + 1], mod_ps[:],
                                     AF.Sigmoid, bias=mb2v_t[:, p:p + 1])

        # ---------- conv + gate + store ----------
        NCH = 8
        for u in range(NU):
            b, p = divmod(u, 2)
            x3 = x_tiles[u][:].rearrange("c (h w) -> c h w", w=WP)
            mod_col = mod_sb[:, 2 * p + b:2 * p + b + 1]
            lt = lts[u]
            for c8 in range(NCH):
                ps = pconv.tile([128, 512], dt.float32, tag="cps", name=f"cps{u}_{c8}")
                for s in range(9):
                    ky, kx = divmod(s, 3)
                    rhs = x3[:, c8 * 8 + ky:c8 * 8 + ky + 8, kx:kx + W]
                    nc.tensor.matmul(ps[:], lt[:, 128 * s:128 * (s + 1)],
                                     rhs, start=(s == 0), stop=(s == 8))
                ot = outpool.tile([128, 512], dt.float32, tag="ot", name=f"ot{u}_{c8}")
                nc.scalar.mul(ot[:], ps[:], mod_col)
                nc.sync.dma_start(
                    y_flat[b * C + 128 * p:b * C + 128 * (p + 1),
                           512 * c8:512 * (c8 + 1)],
                    ot[:])
        ctx.close()

    nc.compile()
    return nc


def _get_nc():
    if "nc" not in _CACHE:
        _CACHE["nc"] = _build_nc()
    return _CACHE["nc"]


def _pad_x(x):
    xp = np.zeros((Bc, C, HP, WP), np.float32)
    xp[:, :, 1:H + 1, 1:W + 1] = x
    return xp


def run_on_device(inputs, trace=False, tmpdir=None):
    """Shard, run on 8 cores, gather. Returns (y_full, BassKernelResults)."""
    from concourse.bass_utils import run_bass_kernel_spmd
    x = np.ascontiguousarray(np.asarray(inputs["x"], np.float32))
    hc = _host_consts(
        inputs["base_kernel"], inputs["ap_w1"], inputs["ap_b1"],
        inputs["ap_w2"], inputs["ap_b2"], inputs["meta_w1"],
        inputs["meta_b1"], inputs["meta_w2"], inputs["meta_b2"])
    nc = _get_nc()
    xpad_full = np.zeros((B, C, HP, WP), np.float32)
    xpad_full[:, :, 1:H + 1, 1:W + 1] = x
    in_maps = []
    for c in range(N_CORES):
        im = {"xs": np.ascontiguousarray(xpad_full[Bc * c:Bc * (c + 1)])}
        im.update(hc)
        in_maps.append(im)
    kw = {}
    if trace:
        kw = dict(trace=True, tmpdir=tmpdir)
    res = run_bass_kernel_spmd(nc, in_maps, core_ids=list(range(N_CORES)), **kw)
    y = np.concatenate([res.results[c]["y"] for c in range(N_CORES)], axis=0)
    return y, res


def kernel(**inputs):
    y, _ = run_on_device(inputs)
    return y


# revision 26
# speedup vs baseline: 1.3934x; 1.3934x over previous
"""Trainium2 Bass kernel for nn_DAC_558345749225 (dynamic rotated grouped conv).

Reference (per sample b):
  pooled = mean_{H,W} x[b]                                  [C]
  angles = tanh(relu(pooled@W1^T+b1)@W2^T+b2) * pi/4        [G]
  rot[g] = bilinear-rotate(base_kernel[g], angles[g])       [Cg,Cg,3,3]
  feat   = grouped_conv3x3(x[b], rot, groups=G, pad=1)
  mod    = sigmoid(relu(pooled@M1^T+bm1)@M2^T+bm2)          [C]
  out    = feat * mod[:,None,None]

Sharding: data-parallel over batch — 2 samples per core on 8 cores.

Per-core pipeline (samples b=0,1; packs p=0,1 of 4 groups each):
  - x is zero-padded to 66x66 on the host and DMAd contiguously into SBUF
    tiles [128ch, 4356] declared float32r (raw fp32 bits; the PE rounds
    internally — verified bitwise-identical to pre-rounded inputs).
  - pooling: free-dim reduce over the padded row (border zeros don't change
    the sum); the 1/4096 scale is folded into the MLP weights host-side.
    Sample 0 reduces on DVE, sample 1 on GpSimd so they run concurrently.
  - per-sample: tiny MLPs on PE + ACT (Relu/Tanh/Sigmoid, cos via
    Sin(x+pi/2)); the 9x9 rotation map R[ij,mn](theta) built with ~22
    elementwise DVE ops on an [8=(g), 81=(mn,ij)] layout.
  - per (b,pack): R scattered into block-diag A [36,36]; one fp32 matmul
      out1[(g,ij),(ci,co)] = A^T @ Bmat   (Bmat = host-rearranged base_kernel)
    out1 rounded to float32r by the ACT copy out of PSUM, then 36 small sync
    DMAs scatter it into block-diag conv weights lhsT [128=(g,ci), 9*128].
  - conv: per (b,pack), 8 chunks of 8 output rows; 9 shift matmuls (float32r,
    1 PE cycle/row, 512 moving rows) accumulate into one PSUM bank; epilogue
    on ACT multiplies by the sigmoid gate (per-partition scale) and the
    result is DMAd to DRAM.
"""
import math
import numpy as np

N_CORES = 8
B, C, H, W = 16, 256, 64, 64
G, Cg = 8, 32
HID = 64
Bc = B // N_CORES          # samples per core = 2
NU = Bc * 2                # (b, pack) units per core = 4
HP, WP = H + 2, W + 2      # padded 66 x 66
NPIX = HP * WP             # 4356

_CACHE = {}


def _host_consts(base_kernel, ap_w1, ap_b1, ap_w2, ap_b2,
                 meta_w1, meta_b1, meta_w2, meta_b2):
    f32 = np.float32
    # Bmat [2, 36, 1024]: [p][g4*9+mn][ci*32+co] = base[4p+g4, co, ci, m, n]
    bk = np.asarray(base_kernel, f32)                      # [G, co, ci, 3, 3]
    bm = np.transpose(bk, (0, 3, 4, 2, 1))                 # [G, m, n, ci, co]
    bmat = np.ascontiguousarray(bm.reshape(2, 36, Cg * Cg))

    # R-build constants over free layout f = mn*9 + ij (mn-major)
    f = np.arange(81)
    mn, ij = f // 9, f % 9
    i, j = ij // 3, ij % 3
    m, n = mn // 3, mn % 3
    blocks = [
        (j - 1).astype(f32),                               # xx
        (i - 1).astype(f32),                               # yy
        n.astype(f32),                                     # nv
        m.astype(f32),                                     # mv
    ]
    consts = np.tile(np.concatenate(blocks)[None, :], (16, 1)).astype(f32)

    scale = f32(1.0 / (H * W))
    w1T = np.asarray(ap_w1, f32).T * scale                 # [256, 64]
    m1T = np.asarray(meta_w1, f32).T * scale
    mlp1 = np.ascontiguousarray(np.concatenate(
        [w1T[:128], w1T[128:], m1T[:128], m1T[128:]], axis=1), f32)  # [128,256]
    w2T = np.asarray(ap_w2, f32).T                         # [64, 8]
    m2T = np.asarray(meta_w2, f32).T                       # [64, 256]
    mlp2 = np.ascontiguousarray(np.concatenate([w2T, m2T], axis=1), f32)
    bias64 = np.ascontiguousarray(
        np.stack([np.asarray(ap_b1, f32), np.asarray(meta_b1, f32)], axis=1))
    b2v = np.asarray(ap_b2, f32).reshape(8, 1).copy()
    mb2v = np.ascontiguousarray(np.asarray(meta_b2, f32).reshape(2, 128).T)
    return dict(bmat=bmat, consts=consts, mlp1=mlp1, mlp2=mlp2,
                bias64=bias64, b2v=b2v, mb2v=mb2v)


def _build_nc():
    import concourse.bass as bass
    import concourse.tile as tile
    from concourse import bacc, mybir
    dt = mybir.dt
    AF = mybir.ActivationFunctionType
    OP = mybir.AluOpType

    nc = bacc.Bacc("TRN2", target_bir_lowering=False, debug=False,
                   enable_asserts=False, num_devices=N_CORES)

    xs = nc.dram_tensor("xs", [Bc, C, HP, WP], dt.float32r, kind="ExternalInput").ap()
    bmat_d = nc.dram_tensor("bmat", [2, 36, 1024], dt.float32, kind="ExternalInput").ap()
    consts_d = nc.dram_tensor("consts", [16, 324], dt.float32, kind="ExternalInput").ap()
    mlp1_d = nc.dram_tensor("mlp1", [128, 256], dt.float32, kind="ExternalInput").ap()
    mlp2_d = nc.dram_tensor("mlp2", [64, 264], dt.float32, kind="ExternalInput").ap()
    bias64_d = nc.dram_tensor("bias64", [64, 2], dt.float32, kind="ExternalInput").ap()
    b2v_d = nc.dram_tensor("b2v", [8, 1], dt.float32, kind="ExternalInput").ap()
    mb2v_d = nc.dram_tensor("mb2v", [128, 2], dt.float32, kind="ExternalInput").ap()
    y = nc.dram_tensor("y", [Bc, C, H, W], dt.float32, kind="ExternalOutput").ap()

    xs_flat = xs.rearrange("b c h w -> (b c) (h w)")
    y_flat = y.rearrange("b c h w -> (b c) (h w)")

    with tile.TileContext(nc) as tc:
        from contextlib import ExitStack
        ctx = ExitStack()
        cpool = ctx.enter_context(tc.tile_pool(name="cpool", bufs=1))
        xpool = ctx.enter_context(tc.tile_pool(name="xpool", bufs=1))
        wpool = ctx.enter_context(tc.tile_pool(name="wpool", bufs=1))
        o1pool = ctx.enter_context(tc.tile_pool(name="o1pool", bufs=2))
        apool = ctx.enter_context(tc.tile_pool(name="apool", bufs=2))
        outpool = ctx.enter_context(tc.tile_pool(name="outpool", bufs=5))
        pconv = ctx.enter_context(tc.tile_pool(name="pconv", bufs=6, space="PSUM"))
        psmall = ctx.enter_context(tc.tile_pool(name="psmall", bufs=2, space="PSUM"))
        dscr = ctx.enter_context(tc.tile_pool(name="dscr", bufs=2, space="DRAM"))

        # ---------- constants ----------
        consts_t = cpool.tile([16, 324], dt.float32)
        nc.sync.dma_start(consts_t[:], consts_d[:])
        XX, YY, NV, MV = (consts_t[:, 81 * k:81 * (k + 1)] for k in range(4))
        mlp1_t = cpool.tile([128, 256], dt.float32)
        nc.sync.dma_start(mlp1_t[:], mlp1_d[:])
        mlp2_t = cpool.tile([64, 264], dt.float32)
        nc.sync.dma_start(mlp2_t[:], mlp2_d[:])
        bias64_t = cpool.tile([64, 2], dt.float32)
        nc.sync.dma_start(bias64_t[:], bias64_d[:])
        b2v_t = cpool.tile([8, 1], dt.float32)
        nc.sync.dma_start(b2v_t[:], b2v_d[:])
        mb2v_t = cpool.tile([128, 2], dt.float32)
        nc.sync.dma_start(mb2v_t[:], mb2v_d[:])
        bsb = cpool.tile([36, 2048], dt.float32)
        nc.sync.dma_start(bsb[:, 0:1024], bmat_d[0])
        nc.sync.dma_start(bsb[:, 1024:2048], bmat_d[1])
        halfpi = cpool.tile([8, 1], dt.float32)
        nc.gpsimd.memset(halfpi[:], math.pi / 2)

        # ---------- x loads (contiguous, two halves per tile) ----------
        # sample 0 loads immediately on sync; sample-1 loads are gated on
        # sample-0 pooling (dummy write) and issued from gpsimd so they don't
        # steal HBM bandwidth from sample 0 or head-of-line-block the sync
        # engine.
        HH = NPIX // 2                 # 2178
        x_tiles = []
        for u in range(NU):
            b, p = divmod(u, 2)
            xt = xpool.tile([128, NPIX], dt.float32r, name=f"xt{u}")
            x_tiles.append(xt)
            if b == 0:
                src = xs_flat[128 * p:128 * (p + 1), :]
                nc.sync.dma_start(xt[:, 0:HH], src[:, 0:HH])
                nc.sync.dma_start(xt[:, HH:NPIX], src[:, HH:NPIX])

        gate_scr = cpool.tile([1, 1], dt.float32)

        def emit_late_xload(u):
            b, p = divmod(u, 2)
            xt = x_tiles[u]
            # gate: dummy READ of the tile (WAR: the DMA write below must wait
            # for it) that also reads sample-0 pooling (RAW: waits for it)
            nc.gpsimd.tensor_scalar(gate_scr[:], xt[0:1, 0:1].bitcast(dt.float32),
                                    pooled[0:1, 2 * (u - 2):2 * (u - 2) + 1],
                                    None, op0=OP.mult)
            src = xs_flat[b * C + 128 * p:b * C + 128 * (p + 1), :]
            nc.gpsimd.dma_start(xt[:, 0:HH], src[:, 0:HH])
            nc.gpsimd.dma_start(xt[:, HH:NPIX], src[:, HH:NPIX])


        # ---------- pooling (emitted per-sample inside the loop below) ----------
        # pooled col layout: col = 2*pack + b
        pooled = cpool.tile([128, NU], dt.float32)
        pp = cpool.tile([128, 2 * NU], dt.float32)

        def emit_pooling(b):
            for p in range(2):
                u = 2 * b + p
                xf = x_tiles[u][:].bitcast(dt.float32)
                nc.vector.reduce_sum(pp[:, 2 * u:2 * u + 1], xf[:, 0:HH],
                                     axis=mybir.AxisListType.X)
                nc.vector.reduce_sum(pp[:, 2 * u + 1:2 * u + 2], xf[:, HH:NPIX],
                                     axis=mybir.AxisListType.X)
                nc.vector.tensor_tensor(pooled[:, 2 * p + b:2 * p + b + 1],
                                        pp[:, 2 * u:2 * u + 1],
                                        pp[:, 2 * u + 1:2 * u + 2], op=OP.add)

        # ---------- per-sample MLPs + R build ----------
        mod_sb = cpool.tile([128, NU], dt.float32)   # col = 2*pack + b

        def vt(nm):
            return cpool.tile([8, 81], dt.float32, name=nm)

        TT = nc.vector.tensor_tensor
        TS = nc.vector.tensor_scalar
        STT = nc.vector.scalar_tensor_tensor

        lts = {}
        for b in range(Bc):
            emit_pooling(b)
            if b == 0:
                emit_late_xload(2)
                emit_late_xload(3)
            # --- angle MLP (this sample only) ---
            h_ps = psmall.tile([64, 1], dt.float32, tag="mlp", name=f"hps{b}")
            nc.tensor.matmul(h_ps[:], mlp1_t[:, 0:64], pooled[:, b:b + 1],
                             start=True, stop=False)
            nc.tensor.matmul(h_ps[:], mlp1_t[:, 64:128], pooled[:, 2 + b:3 + b],
                             start=False, stop=True)
            h_sb = cpool.tile([64, 1], dt.float32, name=f"hsb{b}")
            nc.scalar.activation(h_sb[:], h_ps[:], AF.Relu, bias=bias64_t[:, 0:1])
            ang_ps = psmall.tile([8, 1], dt.float32, tag="mlp", name=f"aps{b}")
            nc.tensor.matmul(ang_ps[:], mlp2_t[:, 0:8], h_sb[:], start=True, stop=True)
            ang_t = cpool.tile([8, 1], dt.float32, name=f"angt{b}")
            nc.scalar.activation(ang_t[:], ang_ps[:], AF.Tanh, bias=b2v_t[:])
            ang_sb = cpool.tile([8, 1], dt.float32, name=f"angs{b}")
            nc.vector.tensor_scalar_mul(ang_sb[:], ang_t[:], math.pi / 4)
            c_sb = cpool.tile([8, 1], dt.float32, name=f"csb{b}")
            nc.scalar.activation(c_sb[:], ang_sb[:], AF.Sin, bias=halfpi[:])
            s_sb = cpool.tile([8, 1], dt.float32, name=f"ssb{b}")
            nc.scalar.activation(s_sb[:], ang_sb[:], AF.Sin)

            # --- R build on an [8=(g), 81=(mn,ij)] layout ---
            # scratch tiles shared across samples (Tile serializes the WAR)
            txc, tys, xr, av, fx, avp = (vt(f"rx{k}") for k in range(6))
            tyc, yr, bv, fy, bvp = (vt(f"ry{k}") for k in range(5))
            u0t, u1t, uu, v0t, v1t, vv = (vt(f"ru{k}") for k in range(6))
            r_b = vt(f"rall_{b}")
            X8, Y8, N8, M8 = XX[0:8], YY[0:8], NV[0:8], MV[0:8]
            cS, sS = c_sb[:], s_sb[:]
            nc.vector.tensor_scalar_mul(txc[:], X8, cS)
            nc.vector.tensor_scalar_mul(tys[:], Y8, sS)
            TT(xr[:], txc[:], tys[:], op=OP.add)
            TS(av[:], xr[:], 0.0, None, op0=OP.is_ge)
            STT(fx[:], xr[:], 1.0, av[:], op0=OP.add, op1=OP.subtract)
            nc.vector.tensor_scalar_add(avp[:], av[:], 1.0)
            nc.vector.tensor_scalar_mul(txc[:], X8, sS)
            nc.vector.tensor_scalar_mul(tyc[:], Y8, cS)
            TT(yr[:], tyc[:], txc[:], op=OP.subtract)
            TS(bv[:], yr[:], 0.0, None, op0=OP.is_ge)
            STT(fy[:], yr[:], 1.0, bv[:], op0=OP.add, op1=OP.subtract)
            nc.vector.tensor_scalar_add(bvp[:], bv[:], 1.0)
            TT(u0t[:], N8, av[:], op=OP.is_equal)
            TT(u1t[:], N8, avp[:], op=OP.is_equal)
            TT(uu[:], u1t[:], u0t[:], op=OP.subtract)
            TT(uu[:], uu[:], fx[:], op=OP.mult)
            TT(uu[:], uu[:], u0t[:], op=OP.add)
            TT(v0t[:], M8, bv[:], op=OP.is_equal)
            TT(v1t[:], M8, bvp[:], op=OP.is_equal)
            TT(vv[:], v1t[:], v0t[:], op=OP.subtract)
            TT(vv[:], vv[:], fy[:], op=OP.mult)
            TT(vv[:], vv[:], v0t[:], op=OP.add)
            TT(r_b[:], uu[:], vv[:], op=OP.mult)

            # --- rotation matmul + weight scatter for this sample's 2 packs ---
            for p in range(2):
                u = 2 * b + p
                a_t = apool.tile([36, 36], dt.float32, tag="a", name=f"a{u}")
                nc.gpsimd.memset(a_t[:], 0.0)
                for g4 in range(4):
                    r = 4 * p + g4
                    nc.sync.dma_start(
                        a_t[:][9 * g4:9 * (g4 + 1), 9 * g4:9 * (g4 + 1)],
                        r_b[:][r:r + 1].rearrange("q (mn ij) -> q mn ij", ij=9))
                o1_t = o1pool.tile([36, 1024], dt.float32r, tag="o1", name=f"o1{u}")
                for hh in range(2):
                    rot_ps = psmall.tile([36, 512], dt.float32, tag="mlp",
                                         name=f"rps{u}{hh}")
                    nc.tensor.matmul(
                        rot_ps[:], a_t[:],
                        bsb[:, 1024 * p + 512 * hh:1024 * p + 512 * (hh + 1)],
                        start=True, stop=True)
                    nc.scalar.copy(o1_t[:, 512 * hh:512 * (hh + 1)], rot_ps[:])
                # bounce out1 through DRAM: DRAM APs have no partition-dim
                # constraints, so the whole (g,ij)x(ci,co) -> (g,ci)x(ij,g,co)
                # rearrangement needs only 1 store + 4 loads
                wd = dscr.tile([36, 1024], dt.float32r, tag="wd", name=f"wd{u}")
                nc.sync.dma_start(wd[:], o1_t[:])
                wv = wd[:].rearrange("(g ij) (ci co) -> g ij ci co", ij=9, co=32)
                lt = wpool.tile([128, 9 * 128], dt.float32r, name=f"lt{u}")
                lts[u] = lt
                nc.gpsimd.memset(lt[:].bitcast(dt.float32), 0.0)
                for g4 in range(4):
                    dst = lt[:][32 * g4:32 * (g4 + 1)].rearrange(
                        "q (ij co) -> q ij co", co=128)[:, :, 32 * g4:32 * (g4 + 1)]
                    nc.sync.dma_start(dst, wv[g4].transpose([1, 0, 2]))

            # --- gate MLP (emitted after rotation; PE order still fine) ---
            m_ps = psmall.tile([64, 1], dt.float32, tag="mlp", name=f"mps{b}")
            nc.tensor.matmul(m_ps[:], mlp1_t[:, 128:192], pooled[:, b:b + 1],
                             start=True, stop=False)
            nc.tensor.matmul(m_ps[:], mlp1_t[:, 192:256], pooled[:, 2 + b:3 + b],
                             start=False, stop=True)
            m_sb = cpool.tile([64, 1], dt.float32, name=f"msb{b}")
            nc.scalar.activation(m_sb[:], m_ps[:], AF.Relu, bias=bias64_t[:, 1:2])
            for p in range(2):
                mod_ps = psmall.tile([128, 1], dt.float32, tag="mlp",
                                     name=f"modps{b}{p}")
                nc.tensor.matmul(mod_ps[:],
                                 mlp2_t[:, 8 + 128 * p:8 + 128 * (p + 1)],
                                 m_sb[:], start=True, stop=True)
                nc.scalar.activation(mod_sb[:, 2 * p + b:2 * p + b + 1], mod_ps[:],
                                     AF.Sigmoid, bias=mb2v_t[:, p:p + 1])

        # ---------- conv + gate + store ----------
        NCH = 8
        for u in range(NU):
            b, p = divmod(u, 2)
            x3 = x_tiles[u][:].rearrange("c (h w) -> c h w", w=WP)
            mod_col = mod_sb[:, 2 * p + b:2 * p + b + 1]
            lt = lts[u]
            for c8 in range(NCH):
                ps = pconv.tile([128, 512], dt.float32, tag="cps", name=f"cps{u}_{c8}")
                for s in range(9):
                    ky, kx = divmod(s, 3)
                    rhs = x3[:, c8 * 8 + ky:c8 * 8 + ky + 8, kx:kx + W]
                    nc.tensor.matmul(ps[:], lt[:, 128 * s:128 * (s + 1)],
                                     rhs, start=(s == 0), stop=(s == 8))
                ot = outpool.tile([128, 512], dt.float32, tag="ot", name=f"ot{u}_{c8}")
                nc.vector.tensor_scalar_mul(ot[:], ps[:], mod_col)
                nc.scalar.dma_start(
                    y_flat[b * C + 128 * p:b * C + 128 * (p + 1),
                           512 * c8:512 * (c8 + 1)],
                    ot[:])
        ctx.close()

    nc.compile()
    return nc


def _get_nc():
    if "nc" not in _CACHE:
        _CACHE["nc"] = _build_nc()
    return _CACHE["nc"]


def _pad_x(x):
    xp = np.zeros((Bc, C, HP, WP), np.float32)
    xp[:, :, 1:H + 1, 1:W + 1] = x
    return xp


def run_on_device(inputs, trace=False, tmpdir=None):
    """Shard, run on 8 cores, gather. Returns (y_full, BassKernelResults)."""
    from concourse.bass_utils import run_bass_kernel_spmd
    x = np.ascontiguousarray(np.asarray(inputs["x"], np.float32))
    hc = _host_consts(
        inputs["base_kernel"], inputs["ap_w1"], inputs["ap_b1"],
        inputs["ap_w2"], inputs["ap_b2"], inputs["meta_w1"],
        inputs["meta_b1"], inputs["meta_w2"], inputs["meta_b2"])
    nc = _get_nc()
    xpad_full = np.zeros((B, C, HP, WP), np.float32)
    xpad_full[:, :, 1:H + 1, 1:W + 1] = x
    in_maps = []
    for c in range(N_CORES):
        im = {"xs": np.ascontiguousarray(xpad_full[Bc * c:Bc * (c + 1)])}
        im.update(hc)
        in_maps.append(im)
    kw = {}
    if trace:
        kw = dict(trace=True, tmpdir=tmpdir)
    res = run_bass_kernel_spmd(nc, in_maps, core_ids=list(range(N_CORES)), **kw)
    y = np.concatenate([res.results[c]["y"] for c in range(N_CORES)], axis=0)
    return y, res


def kernel(**inputs):
    y, _ = run_on_device(inputs)
    return y
